# revision 60
# baseline (speedup 1.0000x reference)
"""BatchedTreeForest (moe_routing) Trainium2 kernel, fp16 edition.

Reference computation (B=4, S=2048, D=1024, O=512, T=16 trees, depth 4):
  logits  = einsum('bsd,tnd->bstn', x, W_dec) + b_dec          (15 internal nodes)
  dec     = sigmoid(logits / softplus(temp_logits + .5413))
  leafp   = prod over the 4 root->leaf path levels of (dec | 1-dec)
  per_tree= einsum('bstl,tlo->bsto', leafp, leaf_outputs)
  gate    = softmax(x @ gate_w + gate_b)
  out     = LayerNorm(einsum('bsto,bst->bso', per_tree, gate)) * gamma + beta

Mapping onto 8 NeuronCores: data-parallel over the 8192 tokens (1024/core),
tree/gate parameters replicated.  All matmul inputs and elementwise
intermediates are fp16 (validated: ~1.8e-3 max rel err vs the 2e-2 gate);
the device also writes fp16 output which the host upcasts.  This halves
HBM traffic in both directions and doubles PE throughput vs fp32.

Per core, 4 "supertiles" of 2x128 tokens, software-pipelined two stages
deep so the PE never waits on the elementwise chain:
  stage A(q): mm1 (16 matmuls -> psum [128, 2, 256] logits), two sigmoid
      ACT ops (scale +-1) -> interleaved [dec, 1-dec] fp16, gate
      e = s/(1-s) on DVE (Z = sum e folded into the layernorm epsilon),
      gate premultiplied into the level-0 pair, 3-stage GpSimd broadcast
      cascade -> gated leaf probs acc [128, 2, 256] fp16
  stage B(q-2): 4 PE transposes -> (t,l)-major, PSUM->SBUF fp16 copy
      split across ACT/DVE, mm2 against leaf_outputs -> psum [128, 512]
      per tile, bn_stats/bn_aggr
  LayerNorm finalize batched (4,2,2 tiles): rsqrt(var + eps*Z^2) via magic
      seed + one Newton step on DVE, one fused (x-mu)*rstd op per tile
      alternating ACT/DVE, fp16 out DMA'd per tile.
"""
import sys

sys.path.insert(0, "/opt/trn_rl_repo")

import numpy as np

P = 128
D = 1024
T = 16
NI = 15
NL = 16
NDEC = T * NI  # 240
COLS = NDEC + T  # 256: decision logits | gate logits
O = 512
NCORES = 8
TOK_PC = 1024  # tokens per core
NTILES = TOK_PC // P  # 8
NST = NTILES // 2  # 4 supertiles of 2 tiles
KT = D // P  # 8 contraction tiles
EPS = 1e-5
MAGIC = 0x5F3759DF


def build(apply_affine: bool = False, has_bias: bool = False):
    """Build the per-core Bass module.  Returns the Bacc object (uncompiled)."""
    import concourse.bacc as bacc
    import concourse.mybir as mybir
    from concourse import masks
    from concourse.tile import TileContext

    f32 = mybir.dt.float32
    f16 = mybir.dt.float16
    i32 = mybir.dt.int32
    Alu = mybir.AluOpType
    Act = mybir.ActivationFunctionType
    X = mybir.AxisListType.X

    nc = bacc.Bacc()
    # Host-prepped layouts (see _host_prep):
    #   xp[j*P+p, k*P+t]   = x[core, j*P+t, k*P+p]   (tile-major, 2KB lines)
    #   wcatp[p, k*COLS+c] = wcat[c, k*P+p]
    #   leafp2[p, c*O+o]   = leaf_outputs[c*P+p, o]
    xp_d = nc.declare_dram_parameter("xp", [NTILES * P, KT * P], f16, isOutput=False)
    wcat_d = nc.declare_dram_parameter("wcatp", [P, KT * COLS], f16, isOutput=False)
    if has_bias:
        bias_d = nc.declare_dram_parameter("biascat", [2, COLS], f16, isOutput=False)
    leaf_d = nc.declare_dram_parameter("leafp2", [P, 2 * O], f16, isOutput=False)
    if apply_affine:
        gamma_d = nc.declare_dram_parameter("gamma", [1, O], f32, isOutput=False)
        beta_d = nc.declare_dram_parameter("beta", [1, O], f32, isOutput=False)
    out_d = nc.declare_dram_parameter("out", [TOK_PC, O], f16, isOutput=True)

    ld = nc.sync

    with TileContext(nc) as tc:
        with (
            tc.tile_pool(name="consts", bufs=1) as consts,
            tc.tile_pool(name="xin", bufs=8) as xin,
            tc.tile_pool(name="d2p", bufs=2) as d2p,
            tc.tile_pool(name="casc", bufs=2) as casc,
            tc.tile_pool(name="accp", bufs=3) as accp,
            tc.tile_pool(name="wtp", bufs=2) as wtp,
            tc.tile_pool(name="outp", bufs=4) as outp,
            tc.tile_pool(name="gatep", bufs=2) as gatep,
            tc.tile_pool(name="smalls", bufs=4) as smalls,
            tc.tile_pool(name="psum1", bufs=2, space="PSUM") as psum1,
            tc.tile_pool(name="psumT", bufs=2, space="PSUM") as psumT,
            tc.tile_pool(name="psum3", bufs=4, space="PSUM") as psum3,
        ):
            # ---- startup loads: the mm1-critical stream (wcat + x tiles 0-3)
            # goes on the sync queue in strict need-order so the wire serves
            # the first tiles first; leaf/bias ride the scalar queue.
            wcat_sb = consts.tile([P, KT, COLS], f16)
            wcat_r = wcat_d[:, :].rearrange("p (k c) -> p k c", k=KT)
            ld.dma_start(out=wcat_sb[:, 0:2], in_=wcat_r[:, 0:2])
            xts = []
            xt0 = xin.tile([P, KT, P], f16, name="xt")
            src0 = xp_d[0:P, :].rearrange("p (k t) -> p k t", k=KT)
            ld.dma_start(out=xt0[:, 0:2], in_=src0[:, 0:2])
            ld.dma_start(out=wcat_sb[:, 2:8], in_=wcat_r[:, 2:8])
            ld.dma_start(out=xt0[:, 2:8], in_=src0[:, 2:8])
            xts.append(xt0)
            for j in range(1, 4):
                xt = xin.tile([P, KT, P], f16, name="xt")
                src = xp_d[j * P : (j + 1) * P, :].rearrange("p (k t) -> p k t", k=KT)
                ld.dma_start(out=xt, in_=src)
                xts.append(xt)
            leaf_sb = consts.tile([P, 2, O], f16)
            nc.scalar.dma_start(
                out=leaf_sb, in_=leaf_d[:, :].rearrange("p (c o) -> p c o", c=2)
            )
            if has_bias:
                bias_sb = consts.tile([1, COLS], f16)
                nc.scalar.dma_start(out=bias_sb, in_=bias_d[0:1, :])
                ones_sb = consts.tile([1, P], f16)
                nc.scalar.dma_start(out=ones_sb, in_=bias_d[1:2, 0:P])
            ident = consts.tile([P, P], f16)
            masks.make_identity(nc, ident[:, :])
            if apply_affine:
                gamma_sb = consts.tile([P, O], f32)
                nc.gpsimd.dma_start(
                    out=gamma_sb, in_=gamma_d[:, :].to_broadcast((P, O))
                )
                beta_sb = consts.tile([P, O], f32)
                nc.gpsimd.dma_start(out=beta_sb, in_=beta_d[:, :].to_broadcast((P, O)))

            # LN finalize batches: a big one mid-flight, small ones at the
            # tail so the post-last-matmul critical path stays short.
            BATCHES = ((0, 1, 2, 3), (4, 5), (6, 7))
            FB = max(len(b) for b in BATCHES)
            batch_of = {}
            for bi, tiles in enumerate(BATCHES):
                for slot, j in enumerate(tiles):
                    batch_of[j] = (bi, slot, tiles)
            state = {}  # per-supertile stage handoffs
            batch = {}
            gz = gatep.tile([P, NTILES], f32, name="gz", tag="gz")

            def stage_a(q):
                """mm1 + decisions + gate + cascade for supertile q."""
                # prefetch x two supertiles ahead on the scalar queue
                for j in (2 * (q + 2), 2 * (q + 2) + 1):
                    if j < NTILES:
                        xt = xin.tile([P, KT, P], f16, name="xt")
                        src = xp_d[j * P : (j + 1) * P, :].rearrange(
                            "p (k t) -> p k t", k=KT
                        )
                        nc.scalar.dma_start(out=xt, in_=src)
                        xts.append(xt)

                # ---- mm1: logits [128, 2 tiles, 240 dec | 16 gate] ----
                ps1 = psum1.tile([P, 2, COLS], f32, name="ps1")
                for a in range(2):
                    xt = xts[2 * q + a]
                    for k in range(KT):
                        nc.tensor.matmul(
                            ps1[:, a], xt[:, k], wcat_sb[:, k], start=(k == 0),
                            stop=(not has_bias and k == KT - 1),
                        )
                    if has_bias:
                        nc.tensor.matmul(
                            ps1[:, a], ones_sb, bias_sb, start=False, stop=True
                        )

                # ---- decisions + gate sigmoids, interleaved [s, 1-s] ----
                d2 = d2p.tile([P, 2, 2 * COLS], f16, name="d2")
                d2v = d2.rearrange("p a (c two) -> p a c two", two=2)
                nc.scalar.activation(out=d2v[:, :, :, 0], in_=ps1, func=Act.Sigmoid)
                nc.scalar.activation(
                    out=d2v[:, :, :, 1], in_=ps1, func=Act.Sigmoid, scale=-1.0
                )

                # ---- gate e = s/(1-s) = exp(logit); Z = sum(e).
                # Z is folded into the layernorm epsilon instead of dividing.
                # decision pairs occupy 30 slots per tree; gates sit at 480+.
                d2t = d2[:, :, 0 : 2 * NDEC].rearrange("p a (t s) -> p a t s", t=T)
                ge = gatep.tile([P, 2, T], f16, name="ge")
                gpair = d2[:, :, 2 * NDEC : 2 * COLS].rearrange(
                    "p a (t two) -> p a t two", two=2
                )
                gr = gatep.tile([P, 2, T], f32, name="gr")
                nc.vector.reciprocal(gr, gpair[:, :, :, 1])
                nc.vector.scalar_tensor_tensor(
                    out=ge, in0=gpair[:, :, :, 0], scalar=1.0, in1=gr,
                    op0=Alu.mult, op1=Alu.mult,
                )
                nc.vector.reduce_sum(gz[:, 2 * q : 2 * q + 2], ge, axis=X)

                # ---- gate premultiplied into the level-0 pair ----
                gpre = casc.tile([P, 2, T, 2], f16, name="gpre")
                nc.gpsimd.tensor_mul(
                    gpre,
                    ge.unsqueeze(-1).broadcast_to((P, 2, T, 2)),
                    d2t[:, :, :, 0:2],
                )

                # ---- cascade: H01 (4/tree) -> H012 (8/tree) -> acc (16/tree)
                # level-d slots of tree t: level1 s=2..5, level2 6..13,
                # level3 14..29 (s = 2*node_in_tree + bit).
                h01 = casc.tile([P, 2, T, 4], f16, name="h01")
                nc.gpsimd.tensor_mul(
                    h01.rearrange("p a t (c r) -> p a t c r", c=2),
                    gpre.unsqueeze(-1).broadcast_to((P, 2, T, 2, 2)),
                    d2t[:, :, :, 2:6].rearrange("p a t (c r) -> p a t c r", c=2),
                )
                h012 = casc.tile([P, 2, T, 8], f16, name="h012")
                nc.gpsimd.tensor_mul(
                    h012.rearrange("p a t (c r) -> p a t c r", c=4),
                    h01.unsqueeze(-1).broadcast_to((P, 2, T, 4, 2)),
                    d2t[:, :, :, 6:14].rearrange("p a t (c r) -> p a t c r", c=4),
                )
                acc = accp.tile([P, 2, T * NL], f16, name="acc")
                nc.gpsimd.tensor_mul(
                    acc.rearrange("p a (t c r) -> p a t c r", t=T, c=8),
                    h012.unsqueeze(-1).broadcast_to((P, 2, T, 8, 2)),
                    d2t[:, :, :, 14:30].rearrange("p a t (c r) -> p a t c r", c=8),
                )
                state[("acc", q)] = acc

            def stage_b(p):
                """transpose + mm2 + LN stats for supertile p."""
                acc = state.pop(("acc", p))
                accf = acc.rearrange("p a tl -> p (a tl)")
                psT = psumT.tile([P, 4, P], f16, name="psT")
                for c in range(4):
                    nc.tensor.transpose(
                        psT[:, c], accf[:, c * P : (c + 1) * P], ident
                    )
                wt = wtp.tile([P, 4, P], f16, name="wt")
                psTf = psT.rearrange("p c t -> p (c t)")
                wtf = wt.rearrange("p c t -> p (c t)")
                nc.scalar.copy(wtf[:, 0 : 2 * P], psTf[:, 0 : 2 * P])
                nc.vector.tensor_copy(wtf[:, 2 * P : 4 * P], psTf[:, 2 * P : 4 * P])

                for a in range(2):
                    j = 2 * p + a
                    bi, jb, tiles = batch_of[j]
                    if jb == 0:
                        batch[bi] = {
                            "mvall": gatep.tile(
                                [P, FB, 2], f32, name="mvall", tag="mvall"
                            ),
                            "ps3s": [],
                        }
                    mvall = batch[bi]["mvall"]
                    ps3 = psum3.tile([P, O], f32, name="ps3")
                    nc.tensor.matmul(
                        ps3, wt[:, 2 * a], leaf_sb[:, 0], start=True, stop=False
                    )
                    nc.tensor.matmul(
                        ps3, wt[:, 2 * a + 1], leaf_sb[:, 1], start=False, stop=True
                    )
                    st6 = smalls.tile([P, 6], f32, name="st6")
                    nc.vector.bn_stats(st6, ps3)
                    nc.vector.bn_aggr(mvall[:, jb], st6)
                    batch[bi]["ps3s"].append((j, ps3))

            def ln_finalize(b):
                """Batched LN finalize for the tiles of BATCHES[b]."""
                tiles = BATCHES[b]
                bsz = len(tiles)
                mvall = batch[b]["mvall"]
                fin_prio = tc.high_priority(offset=120)
                fin_prio.__enter__()
                var4 = mvall[:, 0:bsz, 1]
                mean4 = mvall[:, 0:bsz, 0]
                gzb = gz[:, tiles[0] : tiles[0] + bsz]
                # vt = var + eps*Z^2 ; rstd via magic seed + Newton
                vt = smalls.tile([P, FB], f32, name="vt")[:, 0:bsz]
                nc.vector.scalar_tensor_tensor(
                    out=vt, in0=gzb, scalar=float(EPS), in1=gzb,
                    op0=Alu.mult, op1=Alu.mult,
                )
                nc.vector.tensor_add(vt, vt, var4)
                yt = smalls.tile([P, FB], f32, name="yt")[:, 0:bsz]
                iv = smalls.tile([P, FB], i32, name="iv")[:, 0:bsz]
                nc.vector.tensor_scalar(
                    out=iv, in0=vt.bitcast(i32), scalar1=1, scalar2=None,
                    op0=Alu.logical_shift_right,
                )
                nc.vector.tensor_scalar(
                    out=yt.bitcast(i32), in0=iv, scalar1=-1, scalar2=MAGIC,
                    op0=Alu.mult, op1=Alu.add,
                )
                for _ in range(1):
                    aq = smalls.tile([P, FB], f32, name="aq", tag="aq")[:, 0:bsz]
                    nc.vector.tensor_mul(aq, yt, yt)
                    bq = smalls.tile([P, FB], f32, name="bq", tag="bq")[:, 0:bsz]
                    nc.vector.scalar_tensor_tensor(
                        out=bq, in0=vt, scalar=0.5, in1=aq,
                        op0=Alu.mult, op1=Alu.mult,
                    )
                    cq = smalls.tile([P, FB], f32, name="cq", tag="cq")[:, 0:bsz]
                    nc.vector.tensor_scalar(
                        out=cq, in0=bq, scalar1=-1.0, scalar2=1.5,
                        op0=Alu.mult, op1=Alu.add,
                    )
                    nc.vector.tensor_mul(yt, yt, cq)
                nb = smalls.tile([P, FB], f32, name="nb")[:, 0:bsz]
                nc.vector.scalar_tensor_tensor(
                    out=nb, in0=mean4, scalar=-1.0, in1=yt,
                    op0=Alu.mult, op1=Alu.mult,
                )
                negmu = smalls.tile([P, FB], f32, name="negmu")[:, 0:bsz]
                nc.vector.tensor_scalar(
                    out=negmu, in0=mean4, scalar1=-1.0, scalar2=None, op0=Alu.mult
                )

                # ---- (x - mu) * rstd, one fused op per tile, alternating
                # ACT / DVE; fp16 out DMA'd per tile as soon as it is ready ----
                for qq, (j, ps3) in enumerate(batch[b]["ps3s"]):
                    out_sb = outp.tile([P, O], f16, name="out_sb")
                    if qq % 2 == 0:
                        nc.scalar.activation(
                            out=out_sb, in_=ps3, func=Act.Identity,
                            bias=nb[:, qq : qq + 1], scale=yt[:, qq : qq + 1],
                        )
                    else:
                        nc.vector.tensor_scalar(
                            out=out_sb, in0=ps3,
                            scalar1=negmu[:, qq : qq + 1],
                            scalar2=yt[:, qq : qq + 1],
                            op0=Alu.add, op1=Alu.mult,
                        )
                    if apply_affine:
                        nc.vector.tensor_mul(out_sb, out_sb, gamma_sb)
                        nc.vector.tensor_add(out_sb, out_sb, beta_sb)
                    ld.dma_start(out=out_d[j * P : (j + 1) * P, :], in_=out_sb)
                fin_prio.__exit__(None, None, None)

            # ---- software pipeline, two stages deep ----
            for q in range(NST + 2):
                if q < NST:
                    stage_a(q)
                if q >= 2:
                    p = q - 2
                    stage_b(p)
                    if p == 1:
                        ln_finalize(0)
                    elif p == 2:
                        ln_finalize(1)
                    elif p == 3:
                        ln_finalize(2)

    return nc


def _host_prep(x, decision_weights, decision_biases, leaf_outputs, gate_w, gate_b,
               node_temp_logits, ln_gamma, ln_beta):
    """Fold temperatures into weights/biases, cast to fp16, rearrange into the
    DMA-friendly layouts, shard tokens across the 8 cores."""
    x = np.asarray(x, np.float32)
    temps = np.log1p(np.exp(np.asarray(node_temp_logits, np.float64) + 0.5413))
    temps = temps.astype(np.float32)  # TEMP == 1.0
    wd = (np.asarray(decision_weights, np.float32) / temps[..., None]).reshape(NDEC, D)
    wcat = np.concatenate([wd, np.asarray(gate_w, np.float32).T], axis=0)  # [256, D]
    # wcatp[p, k*COLS + c] = wcat[c, k*P + p]
    wcatp = np.ascontiguousarray(
        wcat.T.reshape(KT, P, COLS).transpose(1, 0, 2).reshape(P, KT * COLS)
    ).astype(np.float16)
    biasrow = np.concatenate(
        [
            (np.asarray(decision_biases, np.float32) / temps).reshape(NDEC),
            np.asarray(gate_b, np.float32),
        ]
    )
    biascat = np.stack([biasrow, np.ones(COLS, np.float32)]).astype(np.float16)
    # leafp2[p, c*O + o] = leaf_outputs[c*P + p, o]
    leafp2 = np.ascontiguousarray(
        np.asarray(leaf_outputs, np.float32)
        .reshape(2, P, O)
        .transpose(1, 0, 2)
        .reshape(P, 2 * O)
    ).astype(np.float16)
    # xp[j*P + p, k*P + t] = xT[k*P + p, j*P + t] per core
    tokens = x.reshape(NCORES, TOK_PC, D)
    xps = []
    for c in range(NCORES):
        xT = tokens[c].T.astype(np.float16)  # [D, TOK_PC]
        xp = (
            xT.reshape(KT, P, NTILES, P)
            .transpose(2, 1, 0, 3)
            .reshape(NTILES * P, KT * P)
        )
        xps.append(np.ascontiguousarray(xp))
    gamma = np.asarray(ln_gamma, np.float32)
    beta = np.asarray(ln_beta, np.float32)
    affine = not (np.all(gamma == 1.0) and np.all(beta == 0.0))
    has_bias = bool(np.any(biasrow != 0.0))
    return xps, wcatp, biascat, leafp2, gamma, beta, affine, has_bias


_BUILT = {}


def _get_module(apply_affine, has_bias):
    key = (apply_affine, has_bias)
    if key not in _BUILT:
        nc = build(apply_affine, has_bias)
        nc.compile()
        _BUILT[key] = nc
    return _BUILT[key]


def run_shards(in_maps, apply_affine=False, has_bias=False, trace=False):
    from concourse.bass_utils import run_bass_kernel_spmd

    nc = _get_module(apply_affine, has_bias)
    return run_bass_kernel_spmd(nc, in_maps, list(range(NCORES)), trace=trace)


def make_in_maps(inputs):
    xps, wcatp, biascat, leafp2, gamma, beta, affine, has_bias = _host_prep(**inputs)
    in_maps = []
    for c in range(NCORES):
        m = {"xp": xps[c], "wcatp": wcatp, "leafp2": leafp2}
        if has_bias:
            m["biascat"] = biascat
        if affine:
            m["gamma"] = gamma[None, :]
            m["beta"] = beta[None, :]
        in_maps.append(m)
    return in_maps, affine, has_bias


def kernel(**inputs) -> np.ndarray:
    B, S = inputs["x"].shape[:2]
    in_maps, affine, has_bias = make_in_maps(inputs)
    res = run_shards(in_maps, apply_affine=affine, has_bias=has_bias)
    out = np.concatenate([res.results[c]["out"] for c in range(NCORES)], axis=0)
    return out.reshape(B, S, O).astype(np.float32)
